# revision 10
# baseline (speedup 1.0000x reference)
"""CoLightNet Trainium2 Bass kernel (self-contained).

SPMD over 8 cores; core c owns output rows [c*1024, (c+1)*1024).
  inputs : stT   [S,N]    bf16  state transposed (host prep)
           stTm  [S,Mc]   bf16  own-rows slice of stT
           adjt  [N,Mc]   fp8e5 adjt[n,m] = adj[row m, col n]  (transposed, 0/1)
           w1,w2,wqk,wh1 [128,128] bf16, wh2 [128,8] bf16
           b1,b2,bh1 [E,1] f32, bh2 [A,1] f32
  output : outb  [Mc,A]   f32

Math (identical to the reference, reformulated):
  hT    = w2^T relu(w1^T stT + b1) + b2                  # [E, N]
  qTp   = Wqk^T hTm            (Wqk = wq wk^T / sqrt(E)) # [E, Mc]
  sT    = hT-block (stationary) x qTp (moving)           # scores^T [N, Mc]
  e     = exp(sT)  (fp8e5);  w = e * adjt (fp8e5)        # post-exp mask
  den   = ones-pair^T x w-pair   (fp8 DoubleRow matmul)  # [1, Mc] in PSUM
  aggT  = h-pair (stationary) x w-pair  (fp8 DoubleRow)  # [E, Mc]
  out   = relu((aggT/den)^T wh1 + bh1) wh2 + bh2

vs previous version: agg+den matmuls use fp8 DoubleRow (2 n-blocks per
pass), hblk produced via DMA-xbar transpose + gpsimd casting DMA instead
of PE transposes, relu/exits on DVE (ACT reserved for exp), phase 1 and
phase 2 interleaved so PE never drains, den tree eliminated.
"""

from contextlib import ExitStack

import concourse.bass as bass
import concourse.mybir as mybir
import concourse.tile as tile
from concourse import bacc
from concourse.masks import make_identity

F32 = mybir.dt.float32
F32R = mybir.dt.float32r
BF16 = mybir.dt.bfloat16
FP8E4 = mybir.dt.float8e4
FP8E5 = mybir.dt.float8e5
AF = mybir.ActivationFunctionType
ALU = mybir.AluOpType
DR = mybir.MatmulPerfMode.DoubleRow

S = 128
E = 128
A = 8

# pairs whose mask-mult runs on GpSimd instead of DVE (load balance)
GP_MASK_PERIOD = 3


def ts(i, size):
    return slice(i * size, (i + 1) * size)


def build_kernel(n_total=8192, m_core=1024):
    nc = bacc.Bacc("TRN2", debug=False)
    stT = nc.dram_tensor("stT", (S, n_total), BF16, kind="ExternalInput").ap()
    stTm = nc.dram_tensor("stTm", (S, m_core), BF16, kind="ExternalInput").ap()
    adjt = nc.dram_tensor("adjt", (n_total, m_core), FP8E5, kind="ExternalInput").ap()
    wt = {}
    for name, shape, dt in [
        ("w1", (S, E), BF16), ("w2", (E, E), BF16), ("wqk", (E, E), BF16),
        ("wh1", (E, E), BF16), ("wh2", (E, A), BF16),
        ("b1", (E, 1), F32), ("b2", (E, 1), F32),
        ("bh1", (E, 1), F32), ("bh2", (A, 1), F32),
    ]:
        wt[name] = nc.dram_tensor(name, shape, dt, kind="ExternalInput").ap()
    outb = nc.dram_tensor("outb", (m_core, A), F32, kind="ExternalOutput").ap()

    with tile.TileContext(nc) as tc:
        colight_body(tc, outb, stT, stTm, adjt, wt)
    nc.compile()
    return nc


def colight_body(tc, outb, stT, stTm, adjt, wt):
    nc = tc.nc
    n_total = stT.shape[1]
    m_core = adjt.shape[1]
    NCH = n_total // 512            # full 512-wide chunks (phase 1)
    NB = n_total // 128             # 128-wide n blocks (phase 2)
    NP = NB // 2                    # block pairs (DoubleRow granularity)
    MH = m_core // 512              # m halves
    NGRP = NB // 4                  # adjacency DMA groups (4 blocks each)
    MCH = m_core // 512             # own chunks

    with ExitStack() as ctx:
        singles = ctx.enter_context(tc.tile_pool(name="singles", bufs=1))

        # ---- staged inputs: big DMAs issued first (deep prefetch) ----
        stTm_sb = singles.tile([128, m_core], BF16, tag="stTm")
        nc.sync.dma_start(out=stTm_sb, in_=stTm)
        stT_sb = singles.tile([128, n_total], BF16, tag="stT")
        for q in range(4):
            nc.sync.dma_start(
                out=stT_sb[:, ts(q, n_total // 4)], in_=stT[:, ts(q, n_total // 4)]
            )

        # ---- constant weights ----
        wf = {}
        for name, shape, dt in [
            ("w1", [S, E], BF16), ("w2", [E, E], BF16), ("wqk", [E, E], BF16),
            ("wh1", [E, E], BF16), ("wh2", [E, A], BF16),
            ("b1", [E, 1], F32), ("b2", [E, 1], F32),
            ("bh1", [E, 1], F32), ("bh2", [A, 1], F32),
        ]:
            t = singles.tile(shape, dt, tag=f"w_{name}")
            nc.scalar.dma_start(out=t, in_=wt[name])
            wf[name] = t
        ones_f = singles.tile([128, 32], F32)
        nc.vector.memset(ones_f, 1.0)
        ones_pair = singles.tile([128, 2, 16], FP8E4)   # den-DR stationary
        nc.vector.tensor_copy(
            out=ones_pair.rearrange("p a b -> p (a b)"), in_=ones_f
        )
        ones_row_f = singles.tile([1, 128], F32)        # head broadcast
        nc.vector.memset(ones_row_f, 1.0)
        ones_row_r = singles.tile([1, 128], F32R)
        nc.vector.tensor_copy(out=ones_row_r, in_=ones_row_f)
        ident_f = singles.tile([128, 128], F32)         # head 8x8 transposes
        make_identity(nc, ident_f)

        # ---- persistent activations ----
        hT = singles.tile([128, NB, 128], BF16)     # h^T, [E, n] (scores lhsT)
        hblk = singles.tile([128, NB, 128], BF16)   # h normal (xbar transpose)
        hpair = singles.tile([128, NB, 128], FP8E4)  # fp8 cast (agg-DR lhsT)
        qTp = singles.tile([128, MH, 512], BF16)    # Wqk^T hTm (scores rhs)
        e_ring = singles.tile([128, 6, m_core], FP8E5)   # exp outputs (ring)

        scp_pool = ctx.enter_context(tc.tile_pool(name="scp", bufs=1, space="PSUM"))
        scp = scp_pool.tile([128, 4, 512], F32)     # scores ring: 4 banks

        # phase-2 SBUF pools (created before the ph1 stack: LIFO discipline)
        adj_pool = ctx.enter_context(tc.tile_pool(name="adj", bufs=4))
        w_pool = ctx.enter_context(tc.tile_pool(name="w", bufs=14))

        # ================= phase 1 pipeline stages =================
        ph1 = ExitStack()
        p1_sb = ph1.enter_context(tc.tile_pool(name="p1_sb", bufs=3))
        p1_ps = ph1.enter_context(tc.tile_pool(name="p1_ps", bufs=2, space="PSUM"))

        # PE warmup: ramp the HAM clock while input DMAs land (results unused)
        wu_sb = p1_sb.tile([128, 512], BF16, tag="wu")
        nc.vector.memset(wu_sb, 1.0)
        for k in range(16):
            wu_ps = p1_ps.tile([128, 512], F32, tag="pa")
            nc.tensor.matmul(wu_ps, wu_sb[:, 0:128], wu_sb, start=True, stop=True)

        ps1s = {}
        h1s = {}
        ps2s = {}

        def src_dma(c):
            if c < MCH:
                return stTm_sb[:, ts(c, 512)]
            return stT_sb[:, ts(c - MCH, 512)]

        def p1_w1(c):
            ps1 = p1_ps.tile([128, 512], F32, tag="pa")
            nc.tensor.matmul(ps1, wf["w1"], src_dma(c), start=True, stop=True)
            ps1s[c] = ps1

        def p1_relu_w2(c):
            ps1 = ps1s.pop(c)
            h1 = p1_sb.tile([128, 512], BF16, tag="h1")
            # h1 = max(ps1 + b1, 0)  (one fused DVE op; ACT is reserved for exp)
            nc.vector.tensor_scalar(
                out=h1, in0=ps1, scalar1=wf["b1"], scalar2=0.0,
                op0=ALU.add, op1=ALU.max,
            )
            ps2 = p1_ps.tile([128, 512], F32, tag="pa")
            nc.tensor.matmul(ps2, wf["w2"], h1, start=True, stop=True)
            ps2s[c] = ps2
            h1s[c] = h1

        def p1_exit(c):
            ps2 = ps2s.pop(c)
            h1s.pop(c)
            if c < MCH:
                hm = p1_sb.tile([128, 512], BF16, tag="hm")
                nc.vector.tensor_scalar(
                    out=hm, in0=ps2, scalar1=wf["b2"], scalar2=None, op0=ALU.add,
                )
                ps3 = p1_ps.tile([128, 512], F32, tag="pa")
                nc.tensor.matmul(ps3, wf["wqk"], hm, start=True, stop=True)
                nc.scalar.activation(qTp[:, c, :], ps3, AF.Copy, bias=0.0, scale=1.0)
            else:
                j = c - MCH
                nc.vector.tensor_scalar(
                    out=hT[:, ts(j, 4), :].rearrange("p a b -> p (a b)"),
                    in0=ps2, scalar1=wf["b2"], scalar2=None, op0=ALU.add,
                )

        def p1_transp(c):
            if c < MCH:
                return
            j = c - MCH
            # hblk[:, b, :][p, e] = h[n = 128*b + p, e]  via DMA xbar transpose
            nc.sync.dma_start_transpose(
                out=hblk[:, ts(j, 4), :],
                in_=hT[:, ts(j, 4), :].rearrange("p a b -> p (a b)"),
            )
            # fp8 cast for the DoubleRow stationary (gpsimd casting DMA)
            nc.gpsimd.dma_start(
                out=hpair[:, ts(j, 4), :], in_=hblk[:, ts(j, 4), :]
            )

        # ================= phase 2 emitters =================
        adj4 = {}
        wbs = {}

        def emit_adj(u):
            a4 = adj_pool.tile([128, 4, m_core], FP8E5, tag="a")
            nc.sync.dma_start(
                out=a4,
                in_=adjt[ts(u, 512), :].rearrange("(a p) m -> p a m", p=128),
            )
            adj4[u] = a4

        def emit_scores(g):
            if g % 4 == 0 and g // 4 + 3 < NGRP:
                emit_adj(g // 4 + 3)
            for j in range(MH):
                nc.tensor.matmul(
                    scp[:, (2 * g + j) % 4, :], hT[:, g, :], qTp[:, j, :],
                    start=True, stop=True,
                )

        def emit_exp(g):
            s0 = (2 * g) % 4
            nc.scalar.activation(
                e_ring[:, g % 6, :],
                scp[:, s0:s0 + 2, :].rearrange("p a b -> p (a b)"),
                AF.Exp, bias=0.0, scale=1.0,
            )

        def emit_mask(p):
            u, v = p // 2, p % 2
            wb = w_pool.tile([128, 2, m_core], FP8E5, tag="w")
            s0 = (2 * p) % 6
            eng = nc.gpsimd if p % GP_MASK_PERIOD == GP_MASK_PERIOD - 1 else nc.vector
            eng.tensor_tensor(
                out=wb.rearrange("p a b -> p (a b)"),
                in0=e_ring[:, s0:s0 + 2, :].rearrange("p a b -> p (a b)"),
                in1=adj4[u][:, 2 * v:2 * v + 2, :].rearrange("p a b -> p (a b)"),
                op=ALU.mult,
            )
            wbs[p] = wb

        # ---- phase-2 emission counters ----
        sc_n = 0   # scores blocks emitted
        ex_n = 0   # exp blocks emitted
        mk_n = 0   # mask pairs emitted

        def ph2_step(drain=False):
            nonlocal sc_n, ex_n, mk_n
            if sc_n < NB:
                emit_scores(sc_n)
                sc_n += 1
            lag = 0 if drain else 1
            while ex_n < sc_n - lag and ex_n < NB:
                emit_exp(ex_n)
                ex_n += 1
            while mk_n < NP and 2 * mk_n + 1 < ex_n:
                emit_mask(mk_n)
                mk_n += 1

        # ---- interleaved phase-1 / phase-2-front (scores+exp+mask only;
        #      agg/den deferred until phase-1 PSUM banks are released) ----
        FRONT_BLOCKS = 12
        for u in range(3):
            emit_adj(u)
        for c in range(MCH):      # own chunks -> qTp (scores need it)
            p1_w1(c)
            p1_relu_w2(c)
            p1_exit(c)

        CTOT = MCH + NCH
        for step in range(MCH, CTOT + 3):
            if step < CTOT:
                p1_w1(step)
            if MCH <= step - 1 < CTOT:
                p1_relu_w2(step - 1)
            if MCH <= step - 2 < CTOT:
                p1_exit(step - 2)
                p1_transp(step - 2)
            limit = min(4 * (step - MCH - 1), FRONT_BLOCKS)
            while sc_n < limit:
                ph2_step()

        ph1.close()

        # ---- phase-2 back: remaining scores/exp/masks + all agg/den ----
        agg_ps_pool = ctx.enter_context(tc.tile_pool(name="agg", bufs=1, space="PSUM"))
        den_ps_pool = ctx.enter_context(tc.tile_pool(name="den", bufs=1, space="PSUM"))
        aggT = agg_ps_pool.tile([128, MH, 512], F32)
        den_ps = den_ps_pool.tile([1, MH, 512], F32)

        def emit_agg_den(p):
            wb = wbs.pop(p)
            for j in range(MH):
                nc.tensor.matmul(
                    aggT[:, j, :], hpair[:, 2 * p:2 * p + 2, :],
                    wb[:, :, ts(j, 512)],
                    start=(p == 0), stop=(p == NP - 1), perf_mode=DR,
                )
            for j in range(MH):
                nc.tensor.matmul(
                    den_ps[:, j, :], ones_pair[:, :, 0:1],
                    wb[:, :, ts(j, 512)],
                    start=(p == 0), stop=(p == NP - 1), perf_mode=DR,
                    skip_group_check=True,
                )

        ag_n = 0

        def drain_agg(nmax):
            nonlocal ag_n
            while ag_n < min(nmax, mk_n):
                emit_agg_den(ag_n)
                ag_n += 1

        while sc_n < NB:
            ph2_step()
            drain_agg(mk_n - 1)
        while ex_n < NB or mk_n < NP:
            ph2_step(drain=True)
            drain_agg(mk_n - 1)
        drain_agg(NP)

        # ================= head =================
        # PSUM is full (scp 4 + agg 2 + den 2); head matmuls reuse scp slices.
        head_sb = ctx.enter_context(tc.tile_pool(name="head_sb", bufs=2))

        # reciprocal of den: [1, m_core] f32
        rscr = head_sb.tile([1, MH * 512], F32, tag="rs")
        rden = head_sb.tile([1, MH, 512], F32, tag="rd")
        nc.vector.reciprocal_approx_accurate(
            out=rden.rearrange("p a b -> p (a b)"),
            in_=den_ps.rearrange("p a b -> p (a b)"),
            scratch=rscr,
        )
        rden_r = head_sb.tile([1, MH, 512], F32R, tag="rdr")
        nc.vector.tensor_copy(
            out=rden_r.rearrange("p a b -> p (a b)"),
            in_=rden.rearrange("p a b -> p (a b)"),
        )
        for j in range(MH):
            rdb_ps = scp[:, 0, :]
            nc.tensor.matmul(rdb_ps, ones_row_r, rden_r[:, j, :], start=True, stop=True)
            rdb = head_sb.tile([128, 512], BF16, tag="rdb")
            nc.vector.tensor_copy(out=rdb, in_=rdb_ps)
            normT = head_sb.tile([128, 512], BF16, tag="n")
            nc.vector.scalar_tensor_tensor(
                out=normT, in0=aggT[:, j, :], scalar=1.0, in1=rdb,
                op0=ALU.mult, op1=ALU.mult,
            )
            h3_ps = scp[:, 1, :]
            nc.tensor.matmul(h3_ps, wf["wh1"], normT, start=True, stop=True)
            h3 = head_sb.tile([128, 512], BF16, tag="h3")
            nc.scalar.activation(h3, h3_ps, AF.Relu, bias=wf["bh1"], scale=1.0)
            oT_ps = scp[0:8, 2, :]
            nc.tensor.matmul(oT_ps, wf["wh2"], h3, start=True, stop=True)
            oT = head_sb.tile([8, 512], F32, tag="oT")
            nc.vector.tensor_scalar(
                out=oT, in0=oT_ps, scalar1=wf["bh2"], scalar2=None, op0=ALU.add,
            )
            o_ps = scp[:, 3, 0:32].rearrange("p (a b) -> p a b", a=4)
            for q in range(4):
                nc.tensor.transpose(o_ps[:, q, :], oT[:, ts(q, 128)], ident_f[0:8, 0:8])
            o_sb = head_sb.tile([128, 4, A], F32, tag="ob")
            nc.vector.tensor_copy(out=o_sb, in_=o_ps)
            nc.scalar.dma_start(
                out=outb[ts(j, 512), :].rearrange("(a p) c -> p a c", p=128),
                in_=o_sb,
            )


# ----------------------------------------------------------------------------
# Host entry point: full inputs in, full output out. 8-way row sharding.
# ----------------------------------------------------------------------------
import numpy as np
import ml_dtypes

N_TOTAL = 8192
N_CORES = 8
M_CORE = N_TOTAL // N_CORES

_cached = {}


def _get_nc():
    if "nc" not in _cached:
        _cached["nc"] = build_kernel(n_total=N_TOTAL, m_core=M_CORE)
    return _cached["nc"]


def _bf16(x):
    return np.ascontiguousarray(
        np.asarray(x, dtype=np.float32).astype(ml_dtypes.bfloat16)
    )


def _fp8e5(x):
    return np.ascontiguousarray(
        np.asarray(x, dtype=np.float32).astype(ml_dtypes.float8_e5m2)
    )


def make_in_maps(state_matrix, adj, w1, b1, w2, b2, wq, wk, wh1, bh1, wh2, bh2):
    f32c = lambda x: np.ascontiguousarray(np.asarray(x, dtype=np.float32))
    stT = _bf16(np.asarray(state_matrix, dtype=np.float32).T)
    wqk = np.asarray(wq, dtype=np.float32) @ np.asarray(wk, dtype=np.float32).T
    wqk = _bf16(wqk / np.float32(np.sqrt(E)))
    adjt_full = _fp8e5(np.asarray(adj).T)
    common = {
        "stT": stT,
        "w1": _bf16(w1), "w2": _bf16(w2), "wqk": wqk,
        "wh1": _bf16(wh1), "wh2": _bf16(wh2),
        "b1": f32c(b1).reshape(E, 1), "b2": f32c(b2).reshape(E, 1),
        "bh1": f32c(bh1).reshape(E, 1), "bh2": f32c(bh2).reshape(A, 1),
    }
    in_maps = []
    for c in range(N_CORES):
        rows = slice(c * M_CORE, (c + 1) * M_CORE)
        in_maps.append(
            dict(
                common,
                stTm=np.ascontiguousarray(stT[:, rows]),
                adjt=np.ascontiguousarray(adjt_full[:, rows]),
            )
        )
    return in_maps


def kernel(state_matrix, adj, w1, b1, w2, b2, wq, wk, wh1, bh1, wh2, bh2):
    from concourse import bass_utils

    in_maps = make_in_maps(
        state_matrix, adj, w1, b1, w2, b2, wq, wk, wh1, bh1, wh2, bh2
    )
    res = bass_utils.run_bass_kernel_spmd(
        _get_nc(), in_maps, core_ids=list(range(N_CORES))
    )
    out = np.concatenate([r["outb"] for r in res.results], axis=0)
    return out.astype(np.float32)
